# revision 7
# baseline (speedup 1.0000x reference)
"""Trainium2 Bass kernel for nn_CrossAttentionBlock (B=2, N=2048, C=1024, H=16).

Sharding: 8 cores; cores 0-3 handle batch 0, cores 4-7 batch 1. Each core owns
a 512-token query slice and computes K/V projections for the FULL batch locally
(no collectives: replicated K/V projection is cheaper than the 8-core
AllGather's ~120us critical-path cost on this part).

On-core layout (bf16 compute, fp32 accumulation):
  qhT, khT   [hd2, tok]  = W^T @ xT   (head pair m at partitions, parity*64)
  vhx        [tok, head, 65] with a ones-column so ctx matmul also yields
             sum(exp) per query
  S^T        [kt, q]     = khT_h^T @ qhT_h; both parities packed concurrently
             in the PE via row tile_position (0,0)/(64,0)
  softmax    exp: scalar-engine exact exp for half the pairs; DVE Schraudolph
             bf16 exp (one fused mult+add -> int16, bitcast bf16) for the rest
  ctx^T      [65, q]     = vhx_h^T @ expS^T accumulated over kt in PSUM
  normalize  reciprocal_approx_fast on sum-row + gpsimd partition_broadcast
  x          [tok, c]    = (sum over ALL head pairs in one PSUM pass) + q + bo
  FFN        y1T [ff, tok] = W1^T @ hT ; gelu ; y2 [tok, c] = gT^T @ W2
"""
import sys

sys.path.insert(0, "/opt/trn_rl_repo")

import numpy as np
import ml_dtypes

import concourse.bass as bass
import concourse.tile as tile
from concourse import bacc, mybir
from concourse.bass_utils import run_bass_kernel_spmd
from concourse.masks import make_identity


def _ensure_ntff_hook():
    """The agent image's antenv package lacks axon_hooks; synthesize it so
    run_bass_kernel_spmd(trace=True) can reach the libaxon NTFF profiler."""
    import types
    if "antenv.axon_hooks" in sys.modules:
        return
    try:
        import antenv
    except ImportError:
        return
    mod = types.ModuleType("antenv.axon_hooks")
    mod._hook = None
    mod.set_axon_ntff_profile_hook = lambda h: setattr(mod, "_hook", h)
    mod.get_axon_ntff_profile_hook = lambda: mod._hook
    sys.modules["antenv.axon_hooks"] = mod
    antenv.axon_hooks = mod
    try:
        from trn_agent_boot.trn_boot import _ntff_profile_via_ctypes
        hook = _ntff_profile_via_ctypes("/opt/axon/libaxon_pjrt.so")
        if hook is not None:
            mod._hook = hook
    except Exception:
        pass


_ensure_ntff_hook()

P = 128
NT = 512          # q-tokens per core
KT = 2048         # keys per batch
B, N, C, H, HD, FF = 2, 2048, 1024, 16, 64, 2048
CT = C // P       # 8 c-tiles
JT = NT // P      # 4 tok-tiles per core
NC = KT // NT     # 4 key chunks (full batch)
IT = KT // P      # 16 kt-tiles
SCALE = HD ** -0.5

# Schraudolph bf16 exp: bits(exp(x)) ~= round(x * 2^7/ln2 + BEXP); uniform
# scale error cancels in softmax normalization, residual spread ~1.8% std.
AEXP = float(2.0 ** 7 / np.log(2.0))
BEXP = 16255.75
# pairs whose exp runs on the scalar engine (exact); others use DVE approx
SCALAR_EXP_PAIRS = (1, 3, 5, 7)

F32 = mybir.dt.float32
F16 = mybir.dt.float16
BF16 = mybir.dt.bfloat16
I16 = mybir.dt.int16
AF = mybir.ActivationFunctionType
ALU = mybir.AluOpType


def build():
    nc = bacc.Bacc(trn_type="TRN2")

    # ---- DRAM parameters (per-core shards; weights replicated) ----
    kT_d = nc.declare_dram_parameter("kT", [NC, P, CT, NT], BF16, isOutput=False)
    vT_d = nc.declare_dram_parameter("vT", [IT, P, CT, P], BF16, isOutput=False)
    qT_d = nc.declare_dram_parameter("qT", [P, CT, NT], BF16, isOutput=False)
    qb_d = nc.declare_dram_parameter("qb", [JT, P, C], BF16, isOutput=False)
    Wk_d = nc.declare_dram_parameter("Wk", [CT, P, CT, P], BF16, isOutput=False)
    Wv_d = nc.declare_dram_parameter("Wv", [P, CT, C], BF16, isOutput=False)
    Wq_d = nc.declare_dram_parameter("Wq", [P, CT, C], BF16, isOutput=False)
    Wo_d = nc.declare_dram_parameter("Wo", [P, CT, C], BF16, isOutput=False)
    W1a_d = nc.declare_dram_parameter("W1a", [P, CT // 2, FF], BF16, isOutput=False)
    W1b_d = nc.declare_dram_parameter("W1b", [P, CT // 2, FF], BF16, isOutput=False)
    W2a_d = nc.declare_dram_parameter("W2a", [P, CT, C], BF16, isOutput=False)
    W2b_d = nc.declare_dram_parameter("W2b", [P, CT, C], BF16, isOutput=False)
    bq_d = nc.declare_dram_parameter("bqt", [P, CT], F32, isOutput=False)
    bk_d = nc.declare_dram_parameter("bkt", [P, CT], F32, isOutput=False)
    b1_d = nc.declare_dram_parameter("b1t", [P, FF // P], F32, isOutput=False)
    bv_d = nc.declare_dram_parameter("bvb", [P, C], BF16, isOutput=False)
    bo_d = nc.declare_dram_parameter("bob", [P, C], BF16, isOutput=False)
    b2_d = nc.declare_dram_parameter("b2b", [P, C], F16, isOutput=False)
    lnw_d = nc.declare_dram_parameter("lnwb", [P, C], BF16, isOutput=False)
    lnb_d = nc.declare_dram_parameter("lnbb", [P, C], BF16, isOutput=False)
    out_d = nc.declare_dram_parameter("out", [NT, C], F32, isOutput=True)

    with tile.TileContext(nc) as tc:
        with (
            tc.tile_pool(name="pers", bufs=1) as pers,
            tc.tile_pool(name="wpool", bufs=1) as wpool,
            tc.tile_pool(name="big", bufs=1) as big,
        ):
            # ---------------- constants / biases ----------------
            ident = pers.tile([P, P], BF16)
            make_identity(nc, ident[:])
            bq_sb = pers.tile([P, CT], F32)
            nc.scalar.dma_start(out=bq_sb[:], in_=bq_d[:])
            bk_sb = pers.tile([P, CT], F32)
            nc.scalar.dma_start(out=bk_sb[:], in_=bk_d[:])
            b1_sb = pers.tile([P, FF // P], F32)
            nc.scalar.dma_start(out=b1_sb[:], in_=b1_d[:])
            eps_sb = pers.tile([P, 1], F32)
            nc.vector.memset(eps_sb[:], 1e-5)

            # ---------------- weights (tag-shared slots) ----------------
            Wk_sb = wpool.tile([P, CT, CT, P], BF16, tag="wB")
            for mm in range(CT):
                nc.scalar.dma_start(out=Wk_sb[:, mm], in_=Wk_d[mm])
            Wv_sb = wpool.tile([P, CT, C], BF16, tag="wC")
            nc.scalar.dma_start(out=Wv_sb[:], in_=Wv_d[:])
            Wq_sb = wpool.tile([P, CT, C], BF16, tag="wA")
            nc.scalar.dma_start(out=Wq_sb[:], in_=Wq_d[:])
            Wo_sb = wpool.tile([P, CT, C], BF16, tag="wD")
            nc.scalar.dma_start(out=Wo_sb[:], in_=Wo_d[:])
            bv_b = pers.tile([P, C], BF16)
            nc.gpsimd.dma_start(out=bv_b[:], in_=bv_d[:])
            bo_b = pers.tile([P, C], BF16)
            nc.gpsimd.dma_start(out=bo_b[:], in_=bo_d[:])
            b2_b = pers.tile([P, C], F16)
            nc.gpsimd.dma_start(out=b2_b[:], in_=b2_d[:])
            lnw_b = pers.tile([P, C], BF16)
            nc.gpsimd.dma_start(out=lnw_b[:], in_=lnw_d[:])
            lnb_b = pers.tile([P, C], BF16)
            nc.gpsimd.dma_start(out=lnb_b[:], in_=lnb_d[:])

            # ---------------- persistent activations ----------------
            x_acc = big.tile([P, JT, C], F16)          # residual accumulator
            qhT3 = big.tile([P, CT, NT], BF16)         # [hd2, q]
            khT3 = big.tile([P, CT, KT], BF16, tag="Tkh")  # [hd2, kt]; reused by gT3
            vhx = big.tile([P, IT, H, HD + 1], BF16, tag="Tvhx")  # reused by hT3

            with (
                tc.tile_pool(name="psA", bufs=1, space="PSUM") as psA,
                tc.tile_pool(name="work", bufs=2) as work,
            ):
                # =========== PHASE A: projections (no collectives) ===========
                # K projection, full batch, chunked over keys
                for n in range(NC):
                    kTn = big.tile([P, CT, NT], BF16, tag="xT", bufs=2,
                                   name=f"kTn{n}")
                    nc.sync.dma_start(out=kTn[:], in_=kT_d[n])
                    for m in range(CT):
                        pk = psA.tile([P, 2, NT], F32, tag="pa", bufs=2,
                                      name=f"pk{n}_{m}")
                        for t in range(CT):
                            nc.tensor.matmul(pk[:, 0, :], Wk_sb[:, m, t, :],
                                             kTn[:, t, :],
                                             start=(t == 0), stop=(t == CT - 1))
                        nc.scalar.activation(
                            out=khT3[:, m, n * NT:(n + 1) * NT], in_=pk[:, 0, :],
                            func=AF.Identity, bias=bk_sb[:, m:m + 1])

                # Q projection + residual init
                qT3 = big.tile([P, CT, NT], BF16, tag="xT", bufs=2)
                nc.sync.dma_start(out=qT3[:], in_=qT_d[:])
                for m in range(CT):
                    pq = psA.tile([P, 2, NT], F32, tag="pa", bufs=2,
                                  name=f"pq{m}")
                    for t in range(CT):
                        nc.tensor.matmul(pq[:, 0, :],
                                         Wq_sb[:, t, m * P:(m + 1) * P],
                                         qT3[:, t, :],
                                         start=(t == 0), stop=(t == CT - 1))
                    nc.vector.tensor_scalar_add(out=qhT3[:, m, :], in0=pq[:, 0, :],
                                                scalar1=bq_sb[:, m:m + 1])
                for j in range(JT):
                    qbj = work.tile([P, C], BF16, tag="hj", bufs=2, name=f"qb{j}")
                    nc.sync.dma_start(out=qbj[:], in_=qb_d[j])
                    nc.vector.tensor_add(out=x_acc[:, j, :], in0=qbj[:], in1=bo_b[:])

                # FFN weights into slots freed by Wq/Wk (Wv/Wo freed later)
                W1a = wpool.tile([P, CT // 2, FF], BF16, tag="wA")
                nc.gpsimd.dma_start(out=W1a[:], in_=W1a_d[:])
                W1b = wpool.tile([P, CT // 2, FF], BF16, tag="wB")
                nc.gpsimd.dma_start(out=W1b[:], in_=W1b_d[:])

                # ======= attention helpers =======
                def emit_scores(pair, i):
                    """Packed scores for both parities of one kt-tile; returns
                    the exp'd tile (bf16 view) [P, 2, NT]."""
                    s_ps = psA.tile([P, 2, NT], F32, tag="pa", bufs=2,
                                    name=f"s{pair}_{i}")
                    for par in range(2):
                        p0 = par * HD
                        nc.tensor.matmul(
                            s_ps[:, par, :],
                            khT3[p0:p0 + HD, pair, i * P:(i + 1) * P],
                            qhT3[p0:p0 + HD, pair, :],
                            start=True, stop=True)
                    if pair in SCALAR_EXP_PAIRS:
                        e_bf = work.tile([P, 2, NT], BF16, tag="expS", bufs=3,
                                         name=f"eb{pair}_{i}")
                        for par in range(2):
                            nc.scalar.activation(out=e_bf[:, par, :],
                                                 in_=s_ps[:, par, :], func=AF.Exp,
                                                 scale=SCALE)
                        return e_bf
                    e_i16 = work.tile([P, 2, NT], I16, tag="expS", bufs=3,
                                      name=f"ei{pair}_{i}")
                    for par in range(2):
                        nc.vector.tensor_scalar(out=e_i16[:, par, :],
                                                in0=s_ps[:, par, :],
                                                scalar1=SCALE * AEXP, scalar2=BEXP,
                                                op0=ALU.mult, op1=ALU.add)
                    return e_i16

                def emit_ctx_mm(pair, i, e, ctx_ps, first, last):
                    for par in range(2):
                        h = 2 * pair + par
                        ev = e[:, par, :]
                        if ev.dtype == I16:
                            ev = ev.bitcast(BF16)
                        nc.tensor.matmul(ctx_ps[:, par, :], vhx[:, i, h, :],
                                         ev, start=first, stop=last)

                def emit_normalize(pair, ctx_ps, stack):
                    """ctx rows 0..63 scaled by 1/row64 -> stack[:, pair, :]."""
                    for par in range(2):
                        dsb = work.tile([1, NT], F32, tag="dsb", bufs=2,
                                        name=f"dsb{pair}_{par}")
                        nc.vector.tensor_copy(out=dsb[:],
                                              in_=ctx_ps[HD:HD + 1, par, :])
                        rc = work.tile([1, NT], F32, tag="rc", bufs=1,
                                       name=f"rc{pair}_{par}")
                        nc.vector.reciprocal_approx_fast(out=rc[:], in_=dsb[:])
                        bc = work.tile([HD, NT], F32, tag="bc", bufs=2,
                                       name=f"bc{pair}_{par}")
                        nc.gpsimd.partition_broadcast(bc[:], rc[:], channels=HD)
                        if par == 0:
                            nc.vector.tensor_mul(out=stack[0:HD, pair, :],
                                                 in0=ctx_ps[0:HD, 0, :], in1=bc[:])
                        else:
                            todd = work.tile([HD, NT], BF16, tag="todd", bufs=2,
                                             name=f"todd{pair}")
                            nc.vector.tensor_mul(out=todd[:],
                                                 in0=ctx_ps[0:HD, 1, :], in1=bc[:])
                            nc.sync.dma_start(out=stack[HD:P, pair, :], in_=todd[:])

                # ======= V projection interleaved with pair-0 attention =======
                stack = big.tile([P, CT, NT], BF16, tag="xT", bufs=2)
                ctx0 = psA.tile([HD + 1, 2, NT], F32, tag="ctx", bufs=2,
                                name="ctx0")
                for i in range(IT):
                    vTc = work.tile([P, CT, P], BF16, tag="vTc", bufs=3,
                                    name=f"vTc{i}")
                    nc.sync.dma_start(out=vTc[:], in_=vT_d[i])
                    pv = psA.tile([P, 2, NT], F32, tag="pa", bufs=2, name=f"pv{i}")
                    for t in range(CT):
                        for n in range(2):
                            nc.tensor.matmul(pv[:, n, :], vTc[:, t, :],
                                             Wv_sb[:, t, n * NT:(n + 1) * NT],
                                             start=(t == 0), stop=(t == CT - 1))
                    nc.vector.tensor_add(
                        out=vhx[:, i, :, 0:HD],
                        in0=pv[:].rearrange("p a b -> p (a b)")
                        .rearrange("p (h d) -> p h d", h=H),
                        in1=bv_b[:].rearrange("p (h d) -> p h d", h=H))
                    nc.gpsimd.memset(vhx[:, i, :, HD:HD + 1], 1.0)
                    e0 = emit_scores(0, i)
                    emit_ctx_mm(0, i, e0, ctx0, first=(i == 0), last=(i == IT - 1))
                emit_normalize(0, ctx0, stack)

                # W2a into the slot freed by Wv
                W2a = wpool.tile([P, CT, C], BF16, tag="wC")
                nc.gpsimd.dma_start(out=W2a[:], in_=W2a_d[:])

                # ======= PHASE B: pairs 1-7 =======
                for pair in range(1, CT):
                    ctx_ps = psA.tile([HD + 1, 2, NT], F32, tag="ctx", bufs=2,
                                      name=f"ctx{pair}")
                    for i in range(IT):
                        e = emit_scores(pair, i)
                        emit_ctx_mm(pair, i, e, ctx_ps,
                                    first=(i == 0), last=(i == IT - 1))
                    emit_normalize(pair, ctx_ps, stack)

                # ======= out-projection: single PSUM pass over all pairs =======
                # W2b trigger first so it rides behind the Wo-free dependency
                W2b = wpool.tile([P, CT, C], BF16, tag="wD")

                # ======= out-proj + LayerNorm + transpose, pipelined per j =======
                hT3 = big.tile([P, CT, NT], BF16, tag="Tvhx")
                mvs = work.tile([P, JT, 2], F32, tag="mvs", bufs=1)
                rstds = work.tile([P, JT], F32, tag="rstds", bufs=1)
                for j in range(JT):
                    op = psA.tile([P, 2, NT], F32, tag="pa", bufs=2,
                                  name=f"op{j}")
                    for n in range(2):
                        for pair in range(CT):
                            nc.tensor.matmul(
                                op[:, n, :], stack[:, pair, j * P:(j + 1) * P],
                                Wo_sb[:, pair, n * NT:(n + 1) * NT],
                                start=(pair == 0), stop=(pair == CT - 1))
                    nc.vector.tensor_add(
                        out=x_acc[:, j, :], in0=x_acc[:, j, :],
                        in1=op[:].rearrange("p a b -> p (a b)"))
                    if j == JT - 1:
                        nc.gpsimd.dma_start(out=W2b[:], in_=W2b_d[:])
                    st = work.tile([P, 2, 6], F32, tag="st", bufs=2, name=f"st{j}")
                    for s in range(2):
                        nc.vector.bn_stats(out=st[:, s, :],
                                           in_=x_acc[:, j, s * NT:(s + 1) * NT])
                    nc.vector.bn_aggr(out=mvs[:, j, :], in_=st[:])
                    nc.scalar.activation(out=rstds[:, j:j + 1], in_=mvs[:, j, 1:2],
                                         func=AF.Sqrt, bias=eps_sb[:])
                    nc.vector.reciprocal(out=rstds[:, j:j + 1],
                                         in_=rstds[:, j:j + 1])
                    hh = work.tile([P, C], F32, tag="hh", bufs=1, name=f"hh{j}")
                    nc.vector.tensor_scalar(out=hh[:], in0=x_acc[:, j, :],
                                            scalar1=mvs[:, j, 0:1],
                                            scalar2=rstds[:, j:j + 1],
                                            op0=ALU.subtract, op1=ALU.mult)
                    nc.vector.tensor_mul(out=hh[:], in0=hh[:], in1=lnw_b[:])
                    hj = work.tile([P, C], BF16, tag="hj", bufs=2, name=f"hj{j}")
                    nc.vector.tensor_add(out=hj[:], in0=hh[:], in1=lnb_b[:])
                    for t in range(CT):
                        tp = psA.tile([P, P], BF16, tag="pa", bufs=2,
                                      name=f"htp{j}_{t}")
                        nc.tensor.transpose(tp[:], hj[:, t * P:(t + 1) * P],
                                            ident[:])
                        nc.scalar.copy(out=hT3[:, t, j * P:(j + 1) * P],
                                       in_=tp[:])

                gT3 = big.tile([P, FF // P, NT], BF16, tag="Tkh")
                for mf in range(FF // P):
                    pf = psA.tile([P, 2, NT], F32, tag="pa", bufs=2,
                                  name=f"pf{mf}")
                    for t in range(CT):
                        wsl = W1a[:, t, mf * P:(mf + 1) * P] if t < 4 else \
                            W1b[:, t - 4, mf * P:(mf + 1) * P]
                        nc.tensor.matmul(pf[:, 0, :], wsl, hT3[:, t, :],
                                         start=(t == 0), stop=(t == CT - 1))
                    nc.scalar.activation(out=gT3[:, mf, :], in_=pf[:, 0, :],
                                         func=AF.Gelu, bias=b1_sb[:, mf:mf + 1])

                for j in range(JT):
                    xb = work.tile([P, C], F16, tag="hh", bufs=1, name=f"xb{j}")
                    nc.vector.tensor_add(out=xb[:], in0=x_acc[:, j, :],
                                         in1=b2_b[:])
                    pf2 = psA.tile([P, 2, NT], F32, tag="ctx", bufs=2,
                                   name=f"pf2_{j}")
                    for n in range(2):
                        for t2 in range(FF // P):
                            w2sl = W2a[:, t2, n * NT:(n + 1) * NT] if t2 < CT \
                                else W2b[:, t2 - CT, n * NT:(n + 1) * NT]
                            nc.tensor.matmul(pf2[:, n, :],
                                             gT3[:, t2, j * P:(j + 1) * P], w2sl,
                                             start=(t2 == 0),
                                             stop=(t2 == FF // P - 1))
                    for n in range(2):
                        out_sb = work.tile([P, NT], F32, tag="osb", bufs=2,
                                           name=f"osb{j}_{n}")
                        nc.vector.tensor_add(out=out_sb[:], in0=pf2[:, n, :],
                                             in1=xb[:, n * NT:(n + 1) * NT])
                        nc.sync.dma_start(
                            out=out_d[j * P:(j + 1) * P, n * NT:(n + 1) * NT],
                            in_=out_sb[:])

    nc.compile()
    return nc


_NC = None
LAST_RESULT = None


def kernel(q, k, v, Wq, bq, Wk, bk, Wv, bv, Wo, bo, ln_w, ln_b, W1, b1, W2, b2):
    global _NC, LAST_RESULT
    if _NC is None:
        _NC = build()
    bf = ml_dtypes.bfloat16

    def wlay(w, rows=None):
        w = np.asarray(w, dtype=bf) if rows is None else np.asarray(w[rows[0]:rows[1]], dtype=bf)
        r, c = w.shape
        return np.ascontiguousarray(w.reshape(r // P, P, c).transpose(1, 0, 2))

    shared = {
        "Wq": wlay(Wq), "Wv": wlay(Wv), "Wo": wlay(Wo),
        "Wk": np.ascontiguousarray(
            np.asarray(Wk, dtype=bf).reshape(CT, P, CT, P).transpose(2, 1, 0, 3)),
        "W1a": wlay(W1, (0, C // 2)), "W1b": wlay(W1, (C // 2, C)),
        "W2a": wlay(W2, (0, C)), "W2b": wlay(W2, (C, FF)),
        "bqt": np.ascontiguousarray(np.asarray(bq, np.float32).reshape(CT, P).T),
        "bkt": np.ascontiguousarray(np.asarray(bk, np.float32).reshape(CT, P).T),
        "b1t": np.ascontiguousarray(np.asarray(b1, np.float32).reshape(FF // P, P).T),
        "bvb": np.ascontiguousarray(np.broadcast_to(np.asarray(bv, bf), (P, C))),
        "bob": np.ascontiguousarray(np.broadcast_to(np.asarray(bo, bf), (P, C))),
        "b2b": np.ascontiguousarray(
            np.broadcast_to(np.asarray(b2, np.float16), (P, C))),
        "lnwb": np.ascontiguousarray(np.broadcast_to(np.asarray(ln_w, bf), (P, C))),
        "lnbb": np.ascontiguousarray(np.broadcast_to(np.asarray(ln_b, bf), (P, C))),
    }
    in_maps = []
    for i in range(8):
        b, r = i // 4, i % 4
        m = dict(shared)
        qs = np.asarray(q[b, r * NT:(r + 1) * NT], np.float32)
        m["qb"] = np.ascontiguousarray(
            qs.astype(bf).reshape(JT, P, C))
        m["qT"] = np.ascontiguousarray(
            qs.T.astype(bf).reshape(CT, P, NT).transpose(1, 0, 2))
        ksh = np.asarray(k[b], np.float32).T.astype(bf)     # [C, KT]
        m["kT"] = np.ascontiguousarray(
            ksh.reshape(CT, P, NC, NT).transpose(2, 1, 0, 3))
        vsh = np.asarray(v[b], np.float32).T.astype(bf)     # [C, KT]
        m["vT"] = np.ascontiguousarray(
            vsh.reshape(CT, P, IT, P).transpose(2, 1, 0, 3))
        in_maps.append(m)
    LAST_RESULT = run_bass_kernel_spmd(_NC, in_maps, core_ids=list(range(8)))
    out = np.empty((B, N, C), np.float32)
    for i in range(8):
        b, r = i // 4, i % 4
        out[b, r * NT:(r + 1) * NT] = LAST_RESULT.results[i]["out"]
    return out


# revision 8
# speedup vs baseline: 1.1746x; 1.1746x over previous
"""Trainium2 Bass kernel for nn_CrossAttentionBlock (B=2, N=2048, C=1024, H=16).

Sharding: 8 cores; cores 0-3 handle batch 0, cores 4-7 batch 1. Each core owns
a 512-token query slice and computes K/V projections for the FULL batch locally
(no collectives: replicated K/V projection is cheaper than the 8-core
AllGather's ~120us critical-path cost on this part).

On-core layout (bf16 compute, fp32 accumulation):
  qhT, khT   [hd2, tok]  = W^T @ xT   (head pair m at partitions, parity*64)
  vhx        [tok, head, 65] with a ones-column so ctx matmul also yields
             sum(exp) per query
  S^T        [kt, q]     = khT_h^T @ qhT_h; both parities packed concurrently
             in the PE via row tile_position (0,0)/(64,0)
  softmax    exp: scalar-engine exact exp for half the pairs; DVE Schraudolph
             bf16 exp (one fused mult+add -> int16, bitcast bf16) for the rest
  ctx^T      [65, q]     = vhx_h^T @ expS^T accumulated over kt in PSUM
  normalize  reciprocal_approx_fast on sum-row + gpsimd partition_broadcast
  x          [tok, c]    = (sum over ALL head pairs in one PSUM pass) + q + bo
  FFN        y1T [ff, tok] = W1^T @ hT ; gelu ; y2 [tok, c] = gT^T @ W2
"""
import sys

sys.path.insert(0, "/opt/trn_rl_repo")

import numpy as np
import ml_dtypes

import concourse.bass as bass
import concourse.tile as tile
from concourse import bacc, mybir
from concourse.bass_utils import run_bass_kernel_spmd
from concourse.masks import make_identity


def _ensure_ntff_hook():
    """The agent image's antenv package lacks axon_hooks; synthesize it so
    run_bass_kernel_spmd(trace=True) can reach the libaxon NTFF profiler."""
    import types
    if "antenv.axon_hooks" in sys.modules:
        return
    try:
        import antenv
    except ImportError:
        return
    mod = types.ModuleType("antenv.axon_hooks")
    mod._hook = None
    mod.set_axon_ntff_profile_hook = lambda h: setattr(mod, "_hook", h)
    mod.get_axon_ntff_profile_hook = lambda: mod._hook
    sys.modules["antenv.axon_hooks"] = mod
    antenv.axon_hooks = mod
    try:
        from trn_agent_boot.trn_boot import _ntff_profile_via_ctypes
        hook = _ntff_profile_via_ctypes("/opt/axon/libaxon_pjrt.so")
        if hook is not None:
            mod._hook = hook
    except Exception:
        pass


_ensure_ntff_hook()

P = 128
NT = 512          # q-tokens per core
KT = 2048         # keys per batch
B, N, C, H, HD, FF = 2, 2048, 1024, 16, 64, 2048
CT = C // P       # 8 c-tiles
JT = NT // P      # 4 tok-tiles per core
NC = KT // NT     # 4 key chunks (full batch)
IT = KT // P      # 16 kt-tiles
SCALE = HD ** -0.5

# Schraudolph bf16 exp: bits(exp(x)) ~= round(x * 2^7/ln2 + BEXP); uniform
# scale error cancels in softmax normalization, residual spread ~1.8% std.
AEXP = float(2.0 ** 7 / np.log(2.0))
BEXP = 16255.75
# pairs whose exp runs on the scalar engine (exact); others use DVE approx
SCALAR_EXP_PAIRS = (1, 3, 5, 7)

F32 = mybir.dt.float32
F16 = mybir.dt.float16
BF16 = mybir.dt.bfloat16
I16 = mybir.dt.int16
AF = mybir.ActivationFunctionType
ALU = mybir.AluOpType


def build():
    nc = bacc.Bacc(trn_type="TRN2")

    # ---- DRAM parameters (per-core shards; weights replicated) ----
    kT_d = nc.declare_dram_parameter("kT", [NC, P, CT, NT], BF16, isOutput=False)
    vT_d = nc.declare_dram_parameter("vT", [IT, P, CT, P], BF16, isOutput=False)
    qT_d = nc.declare_dram_parameter("qT", [P, CT, NT], BF16, isOutput=False)
    qb_d = nc.declare_dram_parameter("qb", [JT, P, C], BF16, isOutput=False)
    Wk_d = nc.declare_dram_parameter("Wk", [CT, P, CT, P], BF16, isOutput=False)
    Wv_d = nc.declare_dram_parameter("Wv", [P, CT, C], BF16, isOutput=False)
    Wq_d = nc.declare_dram_parameter("Wq", [P, CT, C], BF16, isOutput=False)
    Wo_d = nc.declare_dram_parameter("Wo", [P, CT, C], BF16, isOutput=False)
    W1a_d = nc.declare_dram_parameter("W1a", [P, CT // 2, FF], BF16, isOutput=False)
    W1b_d = nc.declare_dram_parameter("W1b", [P, CT // 2, FF], BF16, isOutput=False)
    W2a_d = nc.declare_dram_parameter("W2a", [P, CT, C], BF16, isOutput=False)
    W2b_d = nc.declare_dram_parameter("W2b", [P, CT, C], BF16, isOutput=False)
    bq_d = nc.declare_dram_parameter("bqt", [P, CT], F32, isOutput=False)
    bk_d = nc.declare_dram_parameter("bkt", [P, CT], F32, isOutput=False)
    b1_d = nc.declare_dram_parameter("b1t", [P, FF // P], F32, isOutput=False)
    bv_d = nc.declare_dram_parameter("bvb", [P, C], BF16, isOutput=False)
    bo_d = nc.declare_dram_parameter("bob", [P, C], BF16, isOutput=False)
    b2_d = nc.declare_dram_parameter("b2b", [P, C], F16, isOutput=False)
    lnw_d = nc.declare_dram_parameter("lnwb", [P, C], BF16, isOutput=False)
    lnb_d = nc.declare_dram_parameter("lnbb", [P, C], BF16, isOutput=False)
    out_d = nc.declare_dram_parameter("out", [NT, C], F16, isOutput=True)

    with tile.TileContext(nc) as tc:
        with (
            tc.tile_pool(name="pers", bufs=1) as pers,
            tc.tile_pool(name="wpool", bufs=1) as wpool,
            tc.tile_pool(name="big", bufs=1) as big,
        ):
            # ---------------- constants / biases ----------------
            ident = pers.tile([P, P], BF16)
            make_identity(nc, ident[:])
            bq_sb = pers.tile([P, CT], F32)
            nc.scalar.dma_start(out=bq_sb[:], in_=bq_d[:])
            bk_sb = pers.tile([P, CT], F32)
            nc.scalar.dma_start(out=bk_sb[:], in_=bk_d[:])
            b1_sb = pers.tile([P, FF // P], F32)
            nc.scalar.dma_start(out=b1_sb[:], in_=b1_d[:])
            eps_sb = pers.tile([P, 1], F32)
            nc.vector.memset(eps_sb[:], 1e-5)

            # ---------------- weights (tag-shared slots) ----------------
            Wk_sb = wpool.tile([P, CT, CT, P], BF16, tag="wB")
            for mm in range(CT):
                nc.scalar.dma_start(out=Wk_sb[:, mm], in_=Wk_d[mm])
            Wv_sb = wpool.tile([P, CT, C], BF16, tag="wC")
            nc.scalar.dma_start(out=Wv_sb[:], in_=Wv_d[:])
            Wq_sb = wpool.tile([P, CT, C], BF16, tag="wA")
            nc.scalar.dma_start(out=Wq_sb[:], in_=Wq_d[:])
            Wo_sb = wpool.tile([P, CT, C], BF16, tag="wD")
            nc.scalar.dma_start(out=Wo_sb[:], in_=Wo_d[:])
            bv_b = pers.tile([P, C], BF16)
            nc.gpsimd.dma_start(out=bv_b[:], in_=bv_d[:])
            bo_b = pers.tile([P, C], BF16)
            nc.gpsimd.dma_start(out=bo_b[:], in_=bo_d[:])
            b2_b = pers.tile([P, C], F16)
            nc.gpsimd.dma_start(out=b2_b[:], in_=b2_d[:])
            lnw_b = pers.tile([P, C], BF16)
            nc.gpsimd.dma_start(out=lnw_b[:], in_=lnw_d[:])
            lnb_b = pers.tile([P, C], BF16)
            nc.gpsimd.dma_start(out=lnb_b[:], in_=lnb_d[:])

            # ---------------- persistent activations ----------------
            x_acc = big.tile([P, JT, C], F16)          # residual accumulator
            qhT3 = big.tile([P, CT, NT], BF16)         # [hd2, q]
            khT3 = big.tile([P, CT, KT], BF16, tag="Tkh")  # [hd2, kt]; reused by gT3
            vhx = big.tile([P, IT, H, HD + 1], BF16, tag="Tvhx")  # reused by hT3

            with (
                tc.tile_pool(name="psA", bufs=1, space="PSUM") as psA,
                tc.tile_pool(name="work", bufs=2) as work,
            ):
                # =========== PHASE A: projections (no collectives) ===========
                # K projection, full batch, chunked over keys
                for n in range(NC):
                    kTn = big.tile([P, CT, NT], BF16, tag="xT", bufs=2,
                                   name=f"kTn{n}")
                    nc.sync.dma_start(out=kTn[:], in_=kT_d[n])
                    for m in range(CT):
                        pk = psA.tile([P, 2, NT], F32, tag="pa", bufs=2,
                                      name=f"pk{n}_{m}")
                        for t in range(CT):
                            nc.tensor.matmul(pk[:, 0, :], Wk_sb[:, m, t, :],
                                             kTn[:, t, :],
                                             start=(t == 0), stop=(t == CT - 1))
                        nc.scalar.activation(
                            out=khT3[:, m, n * NT:(n + 1) * NT], in_=pk[:, 0, :],
                            func=AF.Identity, bias=bk_sb[:, m:m + 1])

                # Q projection + residual init
                qT3 = big.tile([P, CT, NT], BF16, tag="xT", bufs=2)
                nc.sync.dma_start(out=qT3[:], in_=qT_d[:])
                for m in range(CT):
                    pq = psA.tile([P, 2, NT], F32, tag="pa", bufs=2,
                                  name=f"pq{m}")
                    for t in range(CT):
                        nc.tensor.matmul(pq[:, 0, :],
                                         Wq_sb[:, t, m * P:(m + 1) * P],
                                         qT3[:, t, :],
                                         start=(t == 0), stop=(t == CT - 1))
                    nc.vector.tensor_scalar_add(out=qhT3[:, m, :], in0=pq[:, 0, :],
                                                scalar1=bq_sb[:, m:m + 1])
                for j in range(JT):
                    qbj = work.tile([P, C], BF16, tag="hj", bufs=2, name=f"qb{j}")
                    nc.sync.dma_start(out=qbj[:], in_=qb_d[j])
                    nc.vector.tensor_add(out=x_acc[:, j, :], in0=qbj[:], in1=bo_b[:])

                # FFN weights into slots freed by Wq/Wk (Wv/Wo freed later)
                W1a = wpool.tile([P, CT // 2, FF], BF16, tag="wA")
                nc.gpsimd.dma_start(out=W1a[:], in_=W1a_d[:])
                W1b = wpool.tile([P, CT // 2, FF], BF16, tag="wB")
                nc.gpsimd.dma_start(out=W1b[:], in_=W1b_d[:])

                # ======= attention helpers =======
                def emit_scores(pair, i):
                    """Packed scores for both parities of one kt-tile; returns
                    the exp'd tile (bf16 view) [P, 2, NT]."""
                    s_ps = psA.tile([P, 2, NT], F32, tag="pa", bufs=2,
                                    name=f"s{pair}_{i}")
                    for par in range(2):
                        p0 = par * HD
                        nc.tensor.matmul(
                            s_ps[:, par, :],
                            khT3[p0:p0 + HD, pair, i * P:(i + 1) * P],
                            qhT3[p0:p0 + HD, pair, :],
                            start=True, stop=True)
                    if pair in SCALAR_EXP_PAIRS:
                        e_bf = work.tile([P, 2, NT], BF16, tag="expS", bufs=3,
                                         name=f"eb{pair}_{i}")
                        nc.scalar.activation(out=e_bf[:], in_=s_ps[:], func=AF.Exp,
                                             scale=SCALE)
                        return e_bf
                    e_i16 = work.tile([P, 2, NT], I16, tag="expS", bufs=3,
                                      name=f"ei{pair}_{i}")
                    nc.vector.tensor_scalar(out=e_i16[:], in0=s_ps[:],
                                            scalar1=SCALE * AEXP, scalar2=BEXP,
                                            op0=ALU.mult, op1=ALU.add)
                    return e_i16

                def emit_ctx_mm(pair, i, e, ctx_ps, first, last):
                    for par in range(2):
                        h = 2 * pair + par
                        ev = e[:, par, :]
                        if ev.dtype == I16:
                            ev = ev.bitcast(BF16)
                        nc.tensor.matmul(ctx_ps[:, par, :], vhx[:, i, h, :],
                                         ev, start=first, stop=last)

                def emit_normalize(pair, ctx_ps, stack):
                    """ctx rows 0..63 scaled by 1/row64 -> stack[:, pair, :]."""
                    dsb = work.tile([1, 2, NT], F32, tag="dsb", bufs=1,
                                    name=f"dsb{pair}")
                    nc.vector.tensor_copy(out=dsb[:], in_=ctx_ps[HD:HD + 1, :, :])
                    rcd = work.tile([1, 2, NT], F32, tag="rc", bufs=1,
                                    name=f"rcd{pair}")
                    nc.vector.reciprocal_approx_fast(out=rcd[:], in_=dsb[:])
                    bc = work.tile([HD, 2, NT], F32, tag="bc", bufs=1,
                                   name=f"bc{pair}")
                    nc.gpsimd.partition_broadcast(bc[:], rcd[:], channels=HD)
                    nc.vector.tensor_mul(out=stack[0:HD, pair, :],
                                         in0=ctx_ps[0:HD, 0, :], in1=bc[:, 0, :])
                    todd = work.tile([HD, NT], BF16, tag="todd", bufs=2,
                                     name=f"todd{pair}")
                    nc.vector.tensor_mul(out=todd[:],
                                         in0=ctx_ps[0:HD, 1, :], in1=bc[:, 1, :])
                    nc.sync.dma_start(out=stack[HD:P, pair, :], in_=todd[:])

                # ======= V projection interleaved with pair-0 attention =======
                stack = big.tile([P, CT, NT], BF16, tag="xT", bufs=2)
                ctx0 = psA.tile([HD + 1, 2, NT], F32, tag="ctx", bufs=2,
                                name="ctx0")
                for i in range(IT):
                    vTc = work.tile([P, CT, P], BF16, tag="vTc", bufs=2,
                                    name=f"vTc{i}")
                    nc.sync.dma_start(out=vTc[:], in_=vT_d[i])
                    pv = psA.tile([P, 2, NT], F32, tag="pa", bufs=2, name=f"pv{i}")
                    for t in range(CT):
                        for n in range(2):
                            nc.tensor.matmul(pv[:, n, :], vTc[:, t, :],
                                             Wv_sb[:, t, n * NT:(n + 1) * NT],
                                             start=(t == 0), stop=(t == CT - 1))
                    nc.vector.tensor_add(
                        out=vhx[:, i, :, 0:HD],
                        in0=pv[:].rearrange("p a b -> p (a b)")
                        .rearrange("p (h d) -> p h d", h=H),
                        in1=bv_b[:].rearrange("p (h d) -> p h d", h=H))
                    nc.gpsimd.memset(vhx[:, i, :, HD:HD + 1], 1.0)
                    e0 = emit_scores(0, i)
                    emit_ctx_mm(0, i, e0, ctx0, first=(i == 0), last=(i == IT - 1))
                emit_normalize(0, ctx0, stack)

                # W2a into the slot freed by Wv
                W2a = wpool.tile([P, CT, C], BF16, tag="wC")
                nc.gpsimd.dma_start(out=W2a[:], in_=W2a_d[:])

                # ======= PHASE B: pairs 1-7, two pairs interleaved so the
                # scalar-exp and DVE-exp streams overlap =======
                for g in ((1, 2), (3, 4), (5, 6), (7,)):
                    ctxs = {p: psA.tile([HD + 1, 2, NT], F32, tag="ctx", bufs=2,
                                        name=f"ctx{p}") for p in g}
                    for i in range(IT):
                        es = [(p, emit_scores(p, i)) for p in g]
                        for p, e in es:
                            emit_ctx_mm(p, i, e, ctxs[p],
                                        first=(i == 0), last=(i == IT - 1))
                    for p in g:
                        emit_normalize(p, ctxs[p], stack)

                # ======= out-projection: single PSUM pass over all pairs =======
                # W2b trigger first so it rides behind the Wo-free dependency
                W2b = wpool.tile([P, CT, C], BF16, tag="wD")

                # ======= out-proj + LayerNorm + transpose, pipelined per j =======
                hT3 = big.tile([P, CT, NT], BF16, tag="Tvhx")
                mvs = work.tile([P, JT, 2], F32, tag="mvs", bufs=1)
                rstds = work.tile([P, JT], F32, tag="rstds", bufs=1)
                hjs = {}

                def emit_transposes(j):
                    for t in range(CT):
                        tp = psA.tile([P, P], BF16, tag="pa", bufs=2,
                                      name=f"htp{j}_{t}")
                        nc.tensor.transpose(tp[:], hjs[j][:, t * P:(t + 1) * P],
                                            ident[:])
                        nc.scalar.copy(out=hT3[:, t, j * P:(j + 1) * P],
                                       in_=tp[:])

                for j in range(JT):
                    op = psA.tile([P, 2, NT], F32, tag="pa", bufs=2,
                                  name=f"op{j}")
                    for n in range(2):
                        for pair in range(CT):
                            nc.tensor.matmul(
                                op[:, n, :], stack[:, pair, j * P:(j + 1) * P],
                                Wo_sb[:, pair, n * NT:(n + 1) * NT],
                                start=(pair == 0), stop=(pair == CT - 1))
                    nc.vector.tensor_add(
                        out=x_acc[:, j, :], in0=x_acc[:, j, :],
                        in1=op[:].rearrange("p a b -> p (a b)"))
                    if j == JT - 1:
                        nc.gpsimd.dma_start(out=W2b[:], in_=W2b_d[:])
                    st = work.tile([P, 2, 6], F32, tag="st", bufs=2, name=f"st{j}")
                    for s in range(2):
                        nc.vector.bn_stats(out=st[:, s, :],
                                           in_=x_acc[:, j, s * NT:(s + 1) * NT])
                    nc.vector.bn_aggr(out=mvs[:, j, :], in_=st[:])
                    nc.scalar.activation(out=rstds[:, j:j + 1], in_=mvs[:, j, 1:2],
                                         func=AF.Sqrt, bias=eps_sb[:])
                    nc.vector.reciprocal(out=rstds[:, j:j + 1],
                                         in_=rstds[:, j:j + 1])
                    hh = work.tile([P, C], F32, tag="hh", bufs=1, name=f"hh{j}")
                    nc.vector.tensor_scalar(out=hh[:], in0=x_acc[:, j, :],
                                            scalar1=mvs[:, j, 0:1],
                                            scalar2=rstds[:, j:j + 1],
                                            op0=ALU.subtract, op1=ALU.mult)
                    nc.vector.tensor_mul(out=hh[:], in0=hh[:], in1=lnw_b[:])
                    hj = hjs[j] = work.tile([P, C], BF16, tag="hj", bufs=2,
                                            name=f"hj{j}")
                    nc.vector.tensor_add(out=hj[:], in0=hh[:], in1=lnb_b[:])
                    if j >= 1:
                        emit_transposes(j - 1)
                emit_transposes(JT - 1)

                gT3 = big.tile([P, FF // P, NT], BF16, tag="Tkh")
                for mf in range(FF // P):
                    pf = psA.tile([P, 2, NT], F32, tag="pa", bufs=2,
                                  name=f"pf{mf}")
                    for t in range(CT):
                        wsl = W1a[:, t, mf * P:(mf + 1) * P] if t < 4 else \
                            W1b[:, t - 4, mf * P:(mf + 1) * P]
                        nc.tensor.matmul(pf[:, 0, :], wsl, hT3[:, t, :],
                                         start=(t == 0), stop=(t == CT - 1))
                    nc.scalar.activation(out=gT3[:, mf, :], in_=pf[:, 0, :],
                                         func=AF.Gelu, bias=b1_sb[:, mf:mf + 1])

                for j in range(JT):
                    xb = work.tile([P, C], F16, tag="hh", bufs=1, name=f"xb{j}")
                    nc.vector.tensor_add(out=xb[:], in0=x_acc[:, j, :],
                                         in1=b2_b[:])
                    pf2 = psA.tile([P, 2, NT], F32, tag="ctx", bufs=2,
                                   name=f"pf2_{j}")
                    for n in range(2):
                        for t2 in range(FF // P):
                            w2sl = W2a[:, t2, n * NT:(n + 1) * NT] if t2 < CT \
                                else W2b[:, t2 - CT, n * NT:(n + 1) * NT]
                            nc.tensor.matmul(pf2[:, n, :],
                                             gT3[:, t2, j * P:(j + 1) * P], w2sl,
                                             start=(t2 == 0),
                                             stop=(t2 == FF // P - 1))
                    for n in range(2):
                        out_sb = work.tile([P, NT], F16, tag="osb", bufs=2,
                                           name=f"osb{j}_{n}")
                        nc.vector.tensor_add(out=out_sb[:], in0=pf2[:, n, :],
                                             in1=xb[:, n * NT:(n + 1) * NT])
                        nc.sync.dma_start(
                            out=out_d[j * P:(j + 1) * P, n * NT:(n + 1) * NT],
                            in_=out_sb[:])

    nc.compile()
    return nc


_NC = None
LAST_RESULT = None


def kernel(q, k, v, Wq, bq, Wk, bk, Wv, bv, Wo, bo, ln_w, ln_b, W1, b1, W2, b2):
    global _NC, LAST_RESULT
    if _NC is None:
        _NC = build()
    bf = ml_dtypes.bfloat16

    def wlay(w, rows=None):
        w = np.asarray(w, dtype=bf) if rows is None else np.asarray(w[rows[0]:rows[1]], dtype=bf)
        r, c = w.shape
        return np.ascontiguousarray(w.reshape(r // P, P, c).transpose(1, 0, 2))

    shared = {
        "Wq": wlay(Wq), "Wv": wlay(Wv), "Wo": wlay(Wo),
        "Wk": np.ascontiguousarray(
            np.asarray(Wk, dtype=bf).reshape(CT, P, CT, P).transpose(2, 1, 0, 3)),
        "W1a": wlay(W1, (0, C // 2)), "W1b": wlay(W1, (C // 2, C)),
        "W2a": wlay(W2, (0, C)), "W2b": wlay(W2, (C, FF)),
        "bqt": np.ascontiguousarray(np.asarray(bq, np.float32).reshape(CT, P).T),
        "bkt": np.ascontiguousarray(np.asarray(bk, np.float32).reshape(CT, P).T),
        "b1t": np.ascontiguousarray(np.asarray(b1, np.float32).reshape(FF // P, P).T),
        "bvb": np.ascontiguousarray(np.broadcast_to(np.asarray(bv, bf), (P, C))),
        "bob": np.ascontiguousarray(np.broadcast_to(np.asarray(bo, bf), (P, C))),
        "b2b": np.ascontiguousarray(
            np.broadcast_to(np.asarray(b2, np.float16), (P, C))),
        "lnwb": np.ascontiguousarray(np.broadcast_to(np.asarray(ln_w, bf), (P, C))),
        "lnbb": np.ascontiguousarray(np.broadcast_to(np.asarray(ln_b, bf), (P, C))),
    }
    in_maps = []
    for i in range(8):
        b, r = i // 4, i % 4
        m = dict(shared)
        qs = np.asarray(q[b, r * NT:(r + 1) * NT], np.float32)
        m["qb"] = np.ascontiguousarray(
            qs.astype(bf).reshape(JT, P, C))
        m["qT"] = np.ascontiguousarray(
            qs.T.astype(bf).reshape(CT, P, NT).transpose(1, 0, 2))
        ksh = np.asarray(k[b], np.float32).T.astype(bf)     # [C, KT]
        m["kT"] = np.ascontiguousarray(
            ksh.reshape(CT, P, NC, NT).transpose(2, 1, 0, 3))
        vsh = np.asarray(v[b], np.float32).T.astype(bf)     # [C, KT]
        m["vT"] = np.ascontiguousarray(
            vsh.reshape(CT, P, IT, P).transpose(2, 1, 0, 3))
        in_maps.append(m)
    LAST_RESULT = run_bass_kernel_spmd(_NC, in_maps, core_ids=list(range(8)))
    out = np.empty((B, N, C), np.float32)
    for i in range(8):
        b, r = i // 4, i % 4
        out[b, r * NT:(r + 1) * NT] = LAST_RESULT.results[i]["out"].astype(np.float32)
    return out
